# revision 30
# baseline (speedup 1.0000x reference)
"""Trainium2 Bass kernel for nn_CharacterModel (char-LSTM + masked sigmoid attention).

Strategy:
  - Data-parallel over words: core c gets sorted words c::8 (lengths stay sorted
    descending per core), 1024 words/core.
  - Ragged packed-sequence processing: at step t only the first n_t words are
    active (n_t = ceil(#global words with len > t / 8), identical across cores).
    Boundary words (ceil rounding) are neutralized via a -60 logit added to the
    attention dot product (sigmoid -> 0); fully-padded chunks skip attention.
  - Layout: hidden/gate dims on partitions, words on the free axis.
    h, c, res are [128, 2, 1024] tiles (hid = j*128 + p).
  - bf16 matmuls (fp8 DoubleRow was tried: rel-err 7e-2, over tolerance).
    Bias enters via a 65th "ones" row of the per-step input matmul. At t=0 h is
    a broadcast h_init, so W_hh@h0+b is folded into the t=0 bias row and the
    recurrent matmul is skipped.
  - Attention per step: the dot product a.h is computed with a REPLICATED
    stationary (attn_rep[:, j, m] = a_half_j for all m), so the PSUM result is
    already broadcast across all 128 partitions; sigmoid runs on [128, cw] and
    res += w*h needs no separate ones-broadcast matmul. The -60 mask matmul is
    emitted only for the chunk containing the true/padded boundary.
  - Host folds the unsort permutation + reshape into the unshard.
"""

import math
import os

import numpy as np
import ml_dtypes

N_WORDS = 8192
MAX_WLEN = 16
EMB = 64
HID = 256
N_SENT = 256
SENT_LEN = 32
NCORES = 8
W = N_WORDS // NCORES  # 1024 words per core
CHUNK = 512

BF16 = ml_dtypes.bfloat16

_BUILD_CACHE = {}
last_result = None  # stashes the most recent BassKernelResults (for profiling)


def _build(nts, nts_true):
    """Build + schedule the Bass program for a given per-step word-count schedule."""
    import concourse.tile as tile
    import concourse.mybir as mybir
    from concourse import bacc

    f32 = mybir.dt.float32
    bf16 = mybir.dt.bfloat16
    AF = mybir.ActivationFunctionType
    OP = mybir.AluOpType

    nc = bacc.Bacc("TRN2", name="char_lstm")

    d_embs = nc.dram_tensor("embs", [MAX_WLEN, EMB + 1, W], bf16, kind="ExternalInput")
    d_wih0 = nc.dram_tensor("wih0", [EMB + 1, 4 * HID], bf16, kind="ExternalInput")
    d_wih = nc.dram_tensor("wih", [EMB + 1, 4 * HID], bf16, kind="ExternalInput")
    d_whh = nc.dram_tensor("whh", [2, 128, 4 * HID], bf16, kind="ExternalInput")
    d_attn = nc.dram_tensor("attn", [128, 2, 128], bf16, kind="ExternalInput")
    d_mask = nc.dram_tensor("masklog", [1, MAX_WLEN * W], bf16, kind="ExternalInput")
    d_c0 = nc.dram_tensor("c0", [128, 2], f32, kind="ExternalInput")
    d_out = nc.dram_tensor("res", [128, 2, W], f32, kind="ExternalOutput")

    GFUNC = [AF.Sigmoid, AF.Sigmoid, AF.Tanh, AF.Sigmoid]  # i, f, g, o

    with tile.TileContext(nc) as tc:
        with (
            tc.tile_pool(name="const", bufs=1) as cp,
            tc.tile_pool(name="embp", bufs=3) as ep,
            tc.tile_pool(name="gatep", bufs=4) as gp,
            tc.tile_pool(name="workp", bufs=6) as wp,
            tc.tile_pool(name="state", bufs=1) as sp,
            tc.tile_pool(name="pgate", bufs=3, space="PSUM") as pg,
            tc.tile_pool(name="pattn", bufs=2, space="PSUM") as pa,
        ):
            # --- constants (critical-path DMAs on sync; bulk on gpsimd queue) ---
            # startup-critical loads: t=0 embeddings + wih0, 3-way split
            # across the DMA-capable queues so the first matmuls feed asap
            # the first x-matmul needs emb0[:, :512] plus the first wih0
            # columns: put exactly those first on their queues
            emb0 = ep.tile([EMB + 1, W], bf16, tag="embt")
            wih0 = cp.tile([EMB + 1, 4 * HID], bf16, tag="wih0")
            nc.sync.dma_start(emb0[:, :512], d_embs[0, :, :512])
            nc.sync.dma_start(wih0[:, :256], d_wih0[:, :256])
            nc.scalar.dma_start(wih0[:, 256:768], d_wih0[:, 256:768])
            nc.scalar.dma_start(emb0[:, 512:768], d_embs[0, :, 512:768])
            nc.gpsimd.dma_start(emb0[:, 768:], d_embs[0, :, 768:])
            nc.gpsimd.dma_start(wih0[:, 768:], d_wih0[:, 768:])
            wih = cp.tile([EMB + 1, 4 * HID], bf16, tag="wih")
            nc.gpsimd.dma_start(wih[:], d_wih[:])
            whh = cp.tile([128, 2, 4 * HID], bf16, tag="whh")
            for k in range(2):
                nc.gpsimd.dma_start(whh[:, k, :], d_whh[k, :, :])
            attn = cp.tile([128, 2, 128], bf16, tag="attn")
            nc.gpsimd.dma_start(attn[:], d_attn[:])
            maskr = cp.tile([1, MAX_WLEN * W], bf16, tag="maskr")
            nc.gpsimd.dma_start(maskr[:], d_mask[:])
            c0t = cp.tile([128, 2], f32, tag="c0")
            nc.gpsimd.dma_start(c0t[:], d_c0[:])
            ones128 = cp.tile([1, 128], bf16, tag="ones128")
            nc.vector.memset(ones128[:], 1.0)

            # --- state ---
            h = sp.tile([128, 2, W], bf16, tag="h")
            c = sp.tile([128, 2, W], bf16, tag="c")
            res = sp.tile([128, 2, W], f32, tag="res")
            nc.vector.memset(res[:], 0.0)

            def emit_attention(at, aw0, awe, antrue):
                # dot broadcast on PE, sigmoid ACT, res += w*h on DVE/GPSIMD
                acw = awe - aw0
                dbc = pa.tile([128, CHUNK], f32, tag="dotbc")
                nc.tensor.matmul(dbc[:, :acw], attn[:, 0, :], h[:, 0, aw0:awe],
                                 start=True, stop=False)
                # ceil-rounding means the word at index n_true-1 may already
                # be finished on most cores: any chunk reaching n_true-1 or
                # beyond needs the -60 mask row.
                if awe < antrue:
                    nc.tensor.matmul(dbc[:, :acw], attn[:, 1, :], h[:, 1, aw0:awe],
                                     start=False, stop=True)
                else:
                    nc.tensor.matmul(dbc[:, :acw], attn[:, 1, :], h[:, 1, aw0:awe],
                                     start=False, stop=False)
                    nc.tensor.matmul(dbc[:, :acw], ones128[:1, :],
                                     maskr[:1, at * W + aw0:at * W + awe],
                                     start=False, stop=True)
                wbc = wp.tile([128, CHUNK], bf16, tag="wrow")
                nc.scalar.activation(wbc[:, :acw], dbc[:, :acw], AF.Sigmoid)
                for j in range(2):
                    t2 = wp.tile([128, CHUNK], bf16, tag="t2")
                    nc.vector.tensor_tensor(t2[:, :acw], h[:, j, aw0:awe], wbc[:, :acw], OP.mult)
                    nc.gpsimd.tensor_tensor(res[:, j, aw0:awe], res[:, j, aw0:awe], t2[:, :acw], OP.add)

            # attention for chunk k is emitted only after chunk k+1's gate
            # matmuls: its PE dot needs h(k) (the end of the ACT/DVE chain),
            # and emitting it immediately would head-of-line-block the PE
            # FIFO at every chunk boundary. Retire-DMAs defer with their
            # step's attention (emission order defines the dependency graph).
            pending_attn = []
            pending_retire = []

            def flush_pending():
                for args in pending_attn:
                    emit_attention(*args)
                pending_attn.clear()
                for lo, hi in pending_retire:
                    nc.sync.dma_start(d_out[:, :, lo:hi], res[:, :, lo:hi])
                pending_retire.clear()

            for t in range(MAX_WLEN):
                n = nts[t]
                if n == 0:
                    break
                n_true = nts_true[t]
                wih_t = wih0 if t == 0 else wih
                if t == 0:
                    embt = emb0  # preloaded above
                else:
                    embt = ep.tile([EMB + 1, W], bf16, tag="embt")
                    nc.sync.dma_start(embt[:, :n], d_embs[t, :, :n])

                # chunking: big steps in 512s; medium steps split in two halves
                # so PE pipelines across the serial ACT/DVE chain; small steps
                # (<=256) run as ONE chunk — the 4 gate tiles already pipeline
                # PE vs ACT, and halving the instruction count beats the lost
                # word-level overlap.
                if n > CHUNK:
                    bounds = list(range(0, n, CHUNK)) + [n]
                elif n > 128:
                    half = (n // 2 + 63) // 64 * 64
                    bounds = [0, half, n]
                else:
                    bounds = [0, n]

                for ci in range(len(bounds) - 1):
                    w0, we = bounds[ci], bounds[ci + 1]
                    cw = we - w0
                    # --- gates (PE -> PSUM, then ACT -> SBUF bf16) ---
                    # x-path matmuls don't depend on h, so they are emitted
                    # FIRST (pairwise): the PE fills the h-wait bubble at step
                    # boundaries with useful work instead of idling (and HAM
                    # down-clocking).
                    gsb = [None] * 4
                    for ga, gb in ((0, 1), (2, 3)):
                        tiles = {}
                        for gi in (ga, gb):
                            ps = pg.tile([128, 2, CHUNK], f32, tag="gates")
                            tiles[gi] = ps
                            for j in range(2):
                                col = gi * 256 + j * 128
                                nc.tensor.matmul(
                                    ps[:, j, :cw], wih_t[:, col:col + 128],
                                    embt[:, w0:we], start=True, stop=(t == 0))
                        for gi in (ga, gb):
                            ps = tiles[gi]
                            if t > 0:
                                for j in range(2):
                                    col = gi * 256 + j * 128
                                    nc.tensor.matmul(
                                        ps[:, j, :cw], whh[:, 0, col:col + 128],
                                        h[:, 0, w0:we], start=False, stop=False)
                                    nc.tensor.matmul(
                                        ps[:, j, :cw], whh[:, 1, col:col + 128],
                                        h[:, 1, w0:we], start=False, stop=True)
                            g_sb = gp.tile([128, 2, CHUNK], bf16, tag=f"g{gi}")
                            nc.scalar.activation(g_sb[:, :, :cw], ps[:, :, :cw], GFUNC[gi])
                            gsb[gi] = g_sb
                    gi_, gf_, gg_, go_ = gsb

                    # previous chunk's attention: its h input is ready, and this
                    # chunk's gate matmuls are already queued ahead of it
                    flush_pending()

                    # --- cell update (DVE) ---
                    # f*c_prev first: it only needs sigma(f) (2nd ACT output),
                    # while i*g waits for tanh(g) (3rd) — better FIFO order
                    ig = wp.tile([128, 2, CHUNK], bf16, tag="ig")
                    if t == 0:
                        nc.vector.tensor_tensor(ig[:, :, :cw], gi_[:, :, :cw], gg_[:, :, :cw], OP.mult)
                        for j in range(2):
                            nc.vector.scalar_tensor_tensor(
                                c[:, j, w0:we], gf_[:, j, :cw], c0t[:, j:j + 1],
                                ig[:, j, :cw], OP.mult, OP.add)
                    else:
                        nc.vector.tensor_tensor(c[:, :, w0:we], gf_[:, :, :cw], c[:, :, w0:we], OP.mult)
                        nc.vector.tensor_tensor(ig[:, :, :cw], gi_[:, :, :cw], gg_[:, :, :cw], OP.mult)
                        nc.vector.tensor_tensor(c[:, :, w0:we], c[:, :, w0:we], ig[:, :, :cw], OP.add)
                    tnc = wp.tile([128, 2, CHUNK], bf16, tag="tanhc")
                    nc.scalar.activation(tnc[:, :, :cw], c[:, :, w0:we], AF.Tanh)
                    nc.vector.tensor_tensor(h[:, :, w0:we], go_[:, :, :cw], tnc[:, :, :cw], OP.mult)

                    if w0 < n_true:  # fully padded chunks skip attention
                        pending_attn.append((t, w0, we, n_true))

                # words [n_{t+1}, n_t) retire after this step: stream them out
                # (deferred until this step's attention has been emitted)
                n_next = nts[t + 1] if t + 1 < MAX_WLEN else 0
                if n_next < n:
                    pending_retire.append((n_next, n))

            flush_pending()
            if nts[MAX_WLEN - 1] > 0:
                nc.sync.dma_start(d_out[:, :, :nts[MAX_WLEN - 1]], res[:, :, :nts[MAX_WLEN - 1]])

    nc.compile()
    return nc


def _get_nc(nts, nts_true):
    key = (tuple(nts), tuple(nts_true))
    if key not in _BUILD_CACHE:
        _BUILD_CACHE[key] = _build(*key)
    return _BUILD_CACHE[key]


def kernel(chars, wordlens, word_orig_idx, emb_table, W_ih, W_hh, b_ih, b_hh,
           attn_w, h_init, c_init):
    global last_result
    from concourse.bass_utils import run_bass_kernel_spmd

    chars = np.asarray(chars)
    wordlens = np.asarray(wordlens)
    word_orig_idx = np.asarray(word_orig_idx)
    emb_table = np.asarray(emb_table, dtype=np.float32)
    W_ih = np.asarray(W_ih, dtype=np.float32)
    W_hh = np.asarray(W_hh, dtype=np.float32)
    b_ih = np.asarray(b_ih, dtype=np.float32)
    b_hh = np.asarray(b_hh, dtype=np.float32)
    attn_w = np.asarray(attn_w, dtype=np.float32)
    h_init = np.asarray(h_init, dtype=np.float32)
    c_init = np.asarray(c_init, dtype=np.float32)

    # per-step active word counts (identical schedule on every core),
    # rounded up to 64 words with a 128-word floor: the rounding words are
    # neutralized by the attention mask (exact result), and the floor keeps
    # enough PE work in the tail that HAM doesn't down-clock it to 4/8
    # (floors 64/192/256 all measured slower).
    nts_true = tuple(int(math.ceil(int((wordlens > t).sum()) / NCORES)) for t in range(MAX_WLEN))
    nts = tuple(max((v + 63) // 64 * 64, 128) if v > 0 else 0 for v in nts_true)
    nc = _get_nc(nts, nts_true)

    bias = b_ih + b_hh
    bias0 = bias + W_hh @ h_init
    wihT = np.concatenate([W_ih.T, bias[None, :]], axis=0)
    wih0T = np.concatenate([W_ih.T, bias0[None, :]], axis=0)
    # attention weight replicated across all 128 stationary columns so the
    # dot product lands broadcast on every PSUM partition
    a2 = attn_w.reshape(2, 128)  # a2[j, p] = a_{j*128+p}
    attn_rep = np.repeat(a2.T[:, :, None], 128, axis=2)  # [128, 2, 128]
    shared = {
        "wih": wihT.astype(BF16),
        "wih0": wih0T.astype(BF16),
        "whh": W_hh.T.reshape(2, 128, 4 * HID).astype(BF16),
        "attn": attn_rep.astype(BF16),
        "c0": c_init.reshape(2, 128).T.copy().astype(np.float32),
    }

    steps = np.arange(MAX_WLEN)[:, None]
    in_maps = []
    for cid in range(NCORES):
        idx = np.arange(W) * NCORES + cid
        embs = emb_table[chars[idx]]            # [W, 16, 64]
        embsT = np.ones((MAX_WLEN, EMB + 1, W), np.float32)
        embsT[:, :EMB, :] = embs.transpose(1, 2, 0)
        lens = wordlens[idx]
        masklog = np.where(lens[None, :] > steps, 0.0, -60.0).astype(np.float32)
        in_maps.append({
            **shared,
            "embs": embsT.astype(BF16),
            "masklog": masklog.reshape(1, -1).astype(BF16),
        })

    last_result = run_bass_kernel_spmd(
        nc, in_maps, core_ids=list(range(NCORES)),
        trace=bool(int(os.environ.get("KERNEL_TRACE", "0"))),
    )

    res_sorted = np.zeros((N_WORDS, HID), np.float32)
    for cid in range(NCORES):
        rc = np.asarray(last_result.results[cid]["res"])  # [128, 2, W]
        res_sorted[np.arange(W) * NCORES + cid] = rc.transpose(2, 1, 0).reshape(W, HID)

    out = np.zeros_like(res_sorted)
    out[word_orig_idx] = res_sorted
    return out.reshape(N_SENT, SENT_LEN, HID)
